# revision 1
# baseline (speedup 1.0000x reference)
"""Trainium2 Bass kernel: per-head attention + residual + LayerNorm.

Problem shape: x [4, 2048, 1024], 16 heads of dk=64, causal softmax attention
with per-head Q/K/V linear projections, residual add, LayerNorm(D).

Sharding (8 cores): head-parallel. Core i owns heads (2i, 2i+1), i.e. feature
columns 128*i : 128*(i+1). Each core computes its feature slice of the output;
the only cross-core communication is a tiny per-batch AllReduce of LayerNorm
partial sums (sum and sum-of-squares over each core's 128 features). The host
shards/gathers and pre-transposes x (the [dk, S] layout each head needs).

Per-core kernel design (bf16 matmuls, fp32 PSUM accumulation):
- Scores via a host-precomputed bilinear form: scores^T = xh_aug^T @ z with
  z = [M @ xh^T + u | beta-row], M = Wk (Wq/sqrt(dk))^T. One projected tensor
  (z) instead of Q and K halves the PSUM->SBUF copies; x^T itself (with a
  built-in ones row for the bias terms) is the stationary matmul operand.
- Flash-style t-outer loop over 1024-col query chunks; scores accumulate in
  PSUM [128,1024] tiles on a 3-deep ring (6 banks) so PE can run two key
  blocks ahead of the exp; exp runs on ScalarE straight from PSUM in one
  instruction per (key-block, chunk), P is bf16 in SBUF.
- Causal mask: an identity-matmul accumulates -40 onto the upper triangle of
  the diagonal 128-block before exp (no vector-engine masking); sub-diagonal
  dead zones are simply never read by PV.
- PV accumulates O directly per 128-query block: lhsT is a 128-col slice of
  the exp'd P tile (keys on partitions), rhs is the ones-augmented V [128,65]
  so softmax denominators ride along as column 64. Each query block's [128,65]
  accumulator lives in a PSUM bank (4 blocks/bank at 128-col stride) across
  the whole key loop, completes at its diagonal key block, and drains early:
  one strided DVE reciprocal per 4 blocks + one fused (O*1/l + x) DVE op per
  block with the LayerNorm row-sum accumulated for free. No PE transposes,
  no O^T staging copies.
- PSUM budget: score ring 3x[128,1024] (6 banks) + a shared 2-slot
  [128,512] ring for O accumulators and z/V projection staging (2 banks).
- Scheduling is convoy-aware: no in-order queue (PE, DVE, ACT, DMA queues)
  may have its head wait on a slow dependency. PV is deferred one key block
  GLOBALLY (crossing chunk boundaries, so the next chunk's scores precede
  the previous chunk's tail PV); per-half LayerNorm stats (sum via the
  drain accum, sumsq folded into the drains) ship to the AllReduce at the
  half's last drain, and the normalize is emitted only at chunk starts once
  the AllReduce is provably back (>=3 drain bursts old). The final half
  ships stats in two 4-tile parts so only a short reduce chain trails the
  loop; large output DMAs are split across the sync and gpsimd queues.
- rstd = exp(-0.5*ln(var+eps)) keeps every activation in one ACT table set
  (natural_log_exp_and_others; enforced by filtering the set map at compile).

Self-contained: hardcodes all shapes; no sibling imports.
"""

import os
import numpy as np
import ml_dtypes

import concourse.bass as bass
import concourse.bacc as bacc
import concourse.mybir as mybir
from concourse.tile import TileContext
from concourse.bass_utils import run_bass_kernel_spmd

B, S, D, H = 4, 2048, 1024, 16
NCORES = 8
HPC = H // NCORES          # heads per core = 2
DK = D // H                # 64
DC = HPC * DK              # 128 feature cols per core
NT = S // 128              # 16 row tiles of 128
EPS = 1e-5
MASKNEG = -40.0
SPBUFS = 3
HOOKJ = 6
PBUFS = 6
OABUFS = 2
R4BUFS = 4
BF = mybir.dt.bfloat16
F32 = mybir.dt.float32
BF_NP = ml_dtypes.bfloat16
RG = [list(range(NCORES))]
A = mybir.AluOpType
AF = mybir.ActivationFunctionType

LAST_RESULTS = None  # BassKernelResults of the last run (for test harness)


def _build_graph(apply_affine: bool, B_: int = B, S_: int = S, rg=None, fake_ar: bool = False) -> bass.Bass:
    nc = bacc.Bacc()
    NT_ = S_ // 128
    if rg is None:
        rg = RG

    xt = nc.declare_dram_parameter("xt", [B_, HPC, DK + 1, S_], BF, isOutput=False)
    xs = nc.declare_dram_parameter("xs", [B_, S_, DC], F32, isOutput=False)
    wpack = nc.declare_dram_parameter(
        "wpack", [DK, HPC * (DK + 1) + HPC * DK], BF, isOutput=False
    )
    zb = nc.declare_dram_parameter("zb", [DK + 1, HPC], F32, isOutput=False)
    bv16 = nc.declare_dram_parameter("bv16", [HPC, 128, 16 * DK], F32, isOutput=False)
    if apply_affine:
        gam = nc.declare_dram_parameter("gam", [128, DC], F32, isOutput=False)
        bet = nc.declare_dram_parameter("bet", [128, DC], F32, isOutput=False)
    out = nc.declare_dram_parameter("out", [B_, S_, DC], F32, isOutput=True)

    # constants baked into the NEFF
    trineg_np = np.where(
        np.arange(128)[:, None] > np.arange(128)[None, :], MASKNEG, 0.0
    ).astype(np.float32)
    imask_h = nc.inline_tensor(
        np.concatenate([np.eye(128, dtype=np.float32), trineg_np], axis=1).astype(
            BF_NP
        ),
        name="imask",
    )

    # collective bounce buffers: LayerNorm stats per (batch, s-half):
    # [tiles-in-half, 2(sum,sumsq), 128 rows]. The very last half ships in
    # two 4-tile parts so only a short reduce chain trails the main loop.
    NHALF = (S_ + 1023) // 1024
    NTH = NT_ // NHALF
    stats_in = nc.dram_tensor("stats_in", [B_, NHALF, NTH, 2, 128], F32)
    stats_out = nc.dram_tensor(
        "stats_out", [B_, NHALF, NTH, 2, 128], F32, addr_space="Shared"
    )

    with TileContext(nc) as tc:
        with (
            tc.tile_pool(name="consts", bufs=1) as cpool,
            tc.tile_pool(name="sb", bufs=2) as sb,
            tc.tile_pool(name="ps", bufs=1, space="PSUM") as ps,
        ):
            # ---- load constants (first-needed first, split across queues
            # so the first score block's inputs land within ~3.5us) ----
            xth0 = [
                sb.tile([DK + 1, S_], BF, tag="xth", name=f"xth0_{h2}", bufs=B_ * HPC)
                for h2 in range(HPC)
            ]
            q4 = S_ // 4
            nc.sync.dma_start(out=xth0[0][:, 0:q4], in_=xt[0, 0, :, 0:q4])
            nc.gpsimd.dma_start(out=xth0[0][:, q4 : 2 * q4], in_=xt[0, 0, :, q4 : 2 * q4])
            wp_t = cpool.tile([DK, HPC * (DK + 1) + HPC * DK], BF, tag="wp")
            nc.sync.dma_start(out=wp_t[:], in_=wpack[:, :])
            zw_t = wp_t[:][:, 0 : HPC * (DK + 1)]
            wv_t = wp_t[:][:, HPC * (DK + 1) : HPC * (DK + 1) + HPC * DK]
            zbq_t = cpool.tile([DK + 1, HPC], F32, tag="zb")
            nc.sync.dma_start(out=zbq_t[:], in_=zb[:, :])
            imaskq_t = cpool.tile([128, 256], BF, tag="imask")
            nc.sync.dma_start(out=imaskq_t[:], in_=imask_h[:, :])
            bv16_t = cpool.tile([128, HPC * 16 * DK], F32, tag="bv16")
            nc.gpsimd.dma_start(out=bv16_t[:, 0 : 16 * DK], in_=bv16[0])
            nc.sync.dma_start(out=xth0[0][:, S_ // 2 : S_], in_=xt[0, 0, :, S_ // 2 : S_])
            nc.sync.dma_start(out=xth0[1][:, 0 : S_ // 2], in_=xt[0, 1, :, 0 : S_ // 2])
            nc.sync.dma_start(out=xth0[1][:, S_ // 2 : S_], in_=xt[0, 1, :, S_ // 2 : S_])
            nc.gpsimd.dma_start(
                out=bv16_t[:, 16 * DK : 32 * DK], in_=bv16[1]
            )
            if apply_affine:
                gam_t = cpool.tile([128, DC], F32, tag="gam")
                nc.sync.dma_start(out=gam_t[:], in_=gam[:, :])
                bet_t = cpool.tile([128, DC], F32, tag="bet")
                nc.sync.dma_start(out=bet_t[:], in_=bet[:, :])

            zb_t = zbq_t
            idn128_t = imaskq_t[:][:, 0:128]
            maskt_t = imaskq_t[:][:, 128:256]
            eps_t = cpool.tile([128, 1], F32, tag="eps")
            nc.vector.memset(eps_t[:], EPS)

            # (drain-seq counter, pending (b, ch, t0p, gn) LN emissions,
            # per-(b,ch) shared stats tile, out-DMA queue alternator,
            # globally deferred PV closure)
            dseq = [0]
            pending_ln = []
            chstate = {}
            lnctr = [0]
            pend_pv = [None]

            def emit_ln(b, ch, t0p, gn, y_b, use_act=False):
                # reduce + normalize tiles [t0p, t0p+gn) of one s-half
                i0 = ch * NTH + t0p
                red = sb.tile([128, 2 * NTH], F32, tag="red", bufs=4)
                nc.sync.dma_start(
                    out=red[:, 0 : 2 * gn].rearrange("p (t c) -> p t c", c=2),
                    in_=stats_out[b, ch, t0p : t0p + gn].rearrange(
                        "t c p -> p t c"
                    ),
                )
                red3 = red[:].rearrange("p (t c) -> p t c", c=2)
                lnctr[0] += 1
                on_pool = lnctr[0] % 2 == 0
                ev = nc.vector
                mean = sb.tile([128, NTH], F32, tag="mean", bufs=4)
                ev.tensor_scalar(
                    mean[:, 0:gn], red3[:, 0:gn, 0], 1.0 / D, None, A.mult
                )
                msq = sb.tile([128, NTH], F32, tag="msq", bufs=4)
                ev.tensor_mul(msq[:, 0:gn], mean[:, 0:gn], mean[:, 0:gn])
                var = sb.tile([128, NTH], F32, tag="var", bufs=4)
                ev.scalar_tensor_tensor(
                    var[:, 0:gn], red3[:, 0:gn, 1], 1.0 / D, msq[:, 0:gn],
                    A.mult, A.subtract,
                )
                lnv = sb.tile([128, NTH], F32, tag="lnv", bufs=4)
                nc.scalar.activation(lnv[:, 0:gn], var[:, 0:gn], AF.Ln, bias=eps_t[:])
                rstd = sb.tile([128, NTH], F32, tag="rstd", bufs=4)
                nc.scalar.activation(rstd[:, 0:gn], lnv[:, 0:gn], AF.Exp, scale=-0.5)
                ostb = sb.tile([128, 128 * NTH], F32, tag="ost", bufs=3)
                if use_act and not apply_affine:
                    # tail path: ACT is idle after the main loop, so the
                    # normalize runs there as Identity(y*rstd - mean*rstd)
                    # (identity shares the exp/ln table set: no reload)
                    nbias = sb.tile([128, NTH], F32, tag="nbias", bufs=4)
                    nc.vector.scalar_tensor_tensor(
                        nbias[:, 0:gn], mean[:, 0:gn], -1.0, rstd[:, 0:gn],
                        A.mult, A.mult,
                    )
                    for k in range(gn):
                        i = i0 + k
                        nc.scalar.activation(
                            ostb[:, 128 * k : 128 * k + 128],
                            y_b[:, 128 * i : 128 * i + 128],
                            AF.Identity,
                            bias=nbias[:, k : k + 1],
                            scale=rstd[:, k : k + 1],
                        )
                else:
                    for k in range(gn):
                        i = i0 + k
                        ev.tensor_scalar(
                            ostb[:, 128 * k : 128 * k + 128],
                            y_b[:, 128 * i : 128 * i + 128],
                            mean[:, k : k + 1],
                            rstd[:, k : k + 1],
                            A.subtract,
                            A.mult,
                        )
                        if apply_affine:
                            ev.tensor_mul(
                                ostb[:, 128 * k : 128 * k + 128],
                                ostb[:, 128 * k : 128 * k + 128],
                                gam_t[:],
                            )
                            ev.tensor_add(
                                ostb[:, 128 * k : 128 * k + 128],
                                ostb[:, 128 * k : 128 * k + 128],
                                bet_t[:],
                            )
                # large outs split across both queues so neither the queue
                # nor the (serial) transfer lane blocks a reduce chain long
                if gn > 4:
                    gh = gn // 2
                    splits = [(0, gh, nc.sync), (gh, gn, nc.gpsimd)]
                else:
                    splits = [(0, gn, nc.gpsimd if on_pool else nc.sync)]
                for s0_, s1_, eng in splits:
                    eng.dma_start(
                        out=out[b, 128 * (i0 + s0_) : 128 * (i0 + s1_), :].rearrange(
                            "(i p) d -> p i d", p=128
                        ),
                        in_=ostb[:, 128 * s0_ : 128 * s1_].rearrange(
                            "p (i d) -> p i d", d=128
                        ),
                    )

            def _pop_ln(min_age=3, use_act=False):
                # only emit an LN whose AllReduce has had >= min_age drain
                # bursts of latency headroom, so queue heads never block
                if pending_ln and dseq[0] - pending_ln[0][0] >= min_age:
                    _, b_, ch_, t0p_, gn_ = pending_ln.pop(0)
                    emit_ln(b_, ch_, t0p_, gn_, y_tiles[b_], use_act=use_act)

            y_tiles = {}
            bstate = {}
            pstate = {}
            pw = min(1024, S_)
            NP = B_ * HPC

            bload = {}

            def emit_loads(b):
                # batch loads prefetched one pair before their projection is
                # emitted, so the projection's z-bias never sits in the DVE
                # queue waiting on a DMA-gated matmul (which would head-block
                # the drains behind it)
                if b == 0 or b in bload:
                    return
                xth = [None, None]
                for h2 in range(HPC):
                    xth[h2] = sb.tile(
                        [DK + 1, S_], BF, tag="xth", name=f"xth{b}_{h2}", bufs=B_ * HPC
                    )
                    nc.sync.dma_start(
                        out=xth[h2][:, 0 : S_ // 2], in_=xt[b, h2, :, 0 : S_ // 2]
                    )
                    nc.sync.dma_start(
                        out=xth[h2][:, S_ // 2 : S_], in_=xt[b, h2, :, S_ // 2 : S_]
                    )
                xs_b = sb.tile([128, S_], F32, tag="xs", name=f"xs{b}")
                hsz = S_ // 2
                for half in range(2):
                    nc.sync.dma_start(
                        out=xs_b[:, hsz * half : hsz * (half + 1)].rearrange(
                            "p (i d) -> p i d", d=128
                        ),
                        in_=xs[b, hsz * half : hsz * (half + 1)].rearrange(
                            "(i p) d -> p i d", p=128
                        ),
                    )
                bload[b] = (xth, xs_b)

            def emit_proj(pair):
                b, hh = divmod(pair, HPC)
                if hh == 0:
                    if b == 0:
                        xth = xth0
                    else:
                        emit_loads(b)  # no-op if already prefetched
                        xth, _ = bload[b]
                    if b in bload:
                        xs_b = bload[b][1]
                    else:
                        xs_b = sb.tile([128, S_], F32, tag="xs", name=f"xs{b}")
                    y_b = sb.tile([128, S_], F32, tag=f"y{b}", name=f"y{b}")
                    y_tiles[b] = y_b
                    bstate[b] = (xth, xs_b, y_b, {})
                    need_xs_dma = b not in bload
                else:
                    need_xs_dma = False
                xth, xs_b, y_b, accs = bstate[b]
                xh = xth[hh]
                # z = [M @ xh^T + u | beta-row]: scores become xh_aug^T @ z
                z = sb.tile([DK + 1, S_], BF, tag="z", name=f"z{pair}", bufs=NP)

                def _zchunk(c):
                    zpt = ps.tile([128, 512], F32, tag="oa", bufs=OABUFS, name=f"zp{c}")
                    zp = zpt[:]
                    nc.tensor.matmul(
                        zp[0 : DK + 1, :],
                        lhsT=zw_t[:, (DK + 1) * hh : (DK + 1) * (hh + 1)],
                        rhs=xh[0:DK, 512 * c : 512 * c + 512],
                        start=True,
                        stop=True,
                    )
                    nc.vector.tensor_scalar(
                        z[:, 512 * c : 512 * c + 512],
                        zp[0 : DK + 1, :],
                        zb_t[:, hh : hh + 1],
                        None,
                        A.add,
                    )

                # V with bias, ones-augmented: v = [V | 1] blocks of 65 cols
                v = sb.tile([128, NT_ * (DK + 1)], BF, tag="v", name=f"v{pair}", bufs=NP)
                v3 = v[:].rearrange("p (t w) -> p t w", w=DK + 1)
                nc.vector.memset(v3[:, :, DK : DK + 1], 1.0)
                gv = min(8, NT_)

                def _vgroup(g):
                    vpt = ps.tile([128, 512], F32, tag="oa", bufs=OABUFS, name=f"vp{g}")
                    vp = vpt[:]
                    for u in range(gv):
                        j = gv * g + u
                        nc.tensor.matmul(
                            vp[:, DK * u : DK * u + DK],
                            lhsT=xh[0:DK, 128 * j : 128 * j + 128],
                            rhs=wv_t[:, hh * DK : hh * DK + DK],
                            start=True,
                            stop=True,
                        )
                    nc.vector.tensor_tensor(
                        v3[:, gv * g : gv * g + gv, 0:DK],
                        vp[:, 0 : gv * DK].rearrange("q (t w) -> q t w", w=DK),
                        bv16_t[:].rearrange("q (h t w) -> q (h t) w", h=HPC, w=DK)[
                            :, hh * 16 : hh * 16 + gv, :
                        ],
                        A.add,
                    )

                # first-chunk prerequisites now (z cols 0:1024, V blocks 0:8
                # only need the first x^T half); the rest is deferred into
                # the pair's own chunk-A loop right after its first scores,
                # so PE never queues behind the second x^T half's DMA
                _zchunk(0)
                _zchunk(1)
                _vgroup(0)

                def _deferred():
                    for c in range(2, S_ // 512):
                        _zchunk(c)
                    for g in range(1, NT_ // gv):
                        _vgroup(g)

                if need_xs_dma:
                    hsz = S_ // 2
                    for half in range(2):
                        nc.sync.dma_start(
                            out=xs_b[:, hsz * half : hsz * (half + 1)].rearrange(
                                "p (i d) -> p i d", d=128
                            ),
                            in_=xs[b, hsz * half : hsz * (half + 1)].rearrange(
                                "(i p) d -> p i d", p=128
                            ),
                        )
                acc_h = sb.tile([128, NT_], F32, tag=f"acc{hh}", name=f"acc{pair}", bufs=B_)
                accs[hh] = acc_h
                pstate[pair] = [xh, z, v3, acc_h, _deferred]

            def emit_chunk(pair, hs, mid_hook=None):
                """Score/exp/PV loop for one 1024-col query chunk. PV writes
                O directly per 128-query block (lhsT = P slice, rhs = V), so
                each block's [128,65] accumulator completes at its diagonal
                key block and drains inline (reciprocal + fused normalize)."""
                b, hh = divmod(pair, HPC)
                xh, z, v3, acc_h, proj_rest = pstate[pair]
                _, xs_b, y_b, accs = bstate[b]
                he = min(S_, hs + 1024)
                w = he - hs
                nblk = w // 128          # query blocks in this chunk
                njb = he // 128          # key blocks (causal: keys < he)
                t0 = hs // 128
                # O accumulators: 4 query blocks per PSUM bank, 128-col stride
                oa = [
                    ps.tile([128, 512], F32, tag="oa", bufs=OABUFS, name=f"oa{g}")
                    for g in range((nblk + 3) // 4)
                ]

                ch = hs // 1024
                last_half = pair == NP - 1 and he == S_
                drain_ends = {3, nblk - 1}

                def _drain(k_lo, k_hi):
                    # blocks [k_lo, k_hi) finished accumulating: 1/l then
                    # y = O*(1/l) + x with LN row-sums accumulated for free
                    gn = k_hi - k_lo
                    gt, off = k_lo // 4, k_lo % 4
                    r4 = sb.tile([128, 4], F32, tag="r4", bufs=R4BUFS)
                    nc.vector.reciprocal(
                        r4[:, 0:gn],
                        oa[gt][:].rearrange("q (k c) -> q k c", c=128)[
                            :, off : off + gn, DK : DK + 1
                        ],
                    )
                    for u in range(gn):
                        k = k_lo + u
                        i = t0 + k
                        nc.vector.scalar_tensor_tensor(
                            y_b[:, 128 * i + DK * hh : 128 * i + DK * hh + DK],
                            oa[gt][:, 128 * (off + u) : 128 * (off + u) + DK],
                            r4[:, u : u + 1],
                            xs_b[:, 128 * i + DK * hh : 128 * i + DK * hh + DK],
                            A.mult,
                            A.add,
                            accum_out=acc_h[:, i : i + 1],
                        )
                    if hh == HPC - 1:
                        # these tiles' y is final for both heads: fold their
                        # LN partial stats into the half's shared tile now
                        if (b, ch) not in chstate:
                            chstate[(b, ch)] = sb.tile(
                                [128, 2 * NTH], F32, tag="st", bufs=3,
                                name=f"st{b}_{ch}",
                            )
                        st = chstate[(b, ch)]
                        # (t c)-interleaved: col 2t = sum, col 2t+1 = sumsq,
                        # matching the t-major DRAM stats layout for the DMA
                        nc.vector.tensor_add(
                            st[:].rearrange("p (t c) -> p t c", c=2)[
                                :, k_lo:k_hi, 0
                            ],
                            accs[0][:, t0 + k_lo : t0 + k_hi],
                            accs[1][:, t0 + k_lo : t0 + k_hi],
                        )
                        for u in range(gn):
                            i = t0 + k_lo + u
                            scr = sb.tile([128, 128], F32, tag="scr")
                            nc.vector.scalar_tensor_tensor(
                                scr[:],
                                y_b[:, 128 * i : 128 * i + 128],
                                1.0,
                                y_b[:, 128 * i : 128 * i + 128],
                                A.mult,
                                A.mult,
                                accum_out=st[
                                    :, 2 * (k_lo + u) + 1 : 2 * (k_lo + u) + 2
                                ],
                            )

                        def _ship(s_lo, s_hi):
                            # stats tiles [s_lo, s_hi) -> AllReduce (sync
                            # queue in, Pool queue reduce) + pend the LN
                            nc.sync.dma_start(
                                out=stats_in[b, ch, s_lo:s_hi].rearrange(
                                    "t c p -> p t c"
                                ),
                                in_=st[:].rearrange("p (t c) -> p t c", c=2)[
                                    :, s_lo:s_hi, :
                                ],
                            )
                            if fake_ar:
                                nc.gpsimd.dma_start(
                                    out=stats_out[b, ch, s_lo:s_hi],
                                    in_=stats_in[b, ch, s_lo:s_hi],
                                )
                            else:
                                nc.gpsimd.collective_compute(
                                    "AllReduce",
                                    A.add,
                                    replica_groups=rg,
                                    ins=[stats_in[b, ch, s_lo:s_hi].opt()],
                                    outs=[stats_out[b, ch, s_lo:s_hi].opt()],
                                )
                            pending_ln.append((dseq[0], b, ch, s_lo, s_hi - s_lo))

                        if last_half:
                            _ship(k_lo, k_hi)  # per-part: shorter tail chain
                        elif k_hi == nblk:
                            _ship(0, nblk)
                    dseq[0] += 1

                for j in range(njb):
                    s0 = 128 * j
                    rel = s0 - hs
                    p = sb.tile([128, 1024], BF, tag="p", bufs=PBUFS)
                    sp = ps.tile([128, 1024], F32, tag="sp", bufs=SPBUFS)
                    if rel < 0:
                        ss = 0
                        while ss < w:
                            sl = min(512, w - ss)
                            nc.tensor.matmul(
                                sp[:, ss : ss + sl],
                                lhsT=xh[:, s0 : s0 + 128],
                                rhs=z[:, hs + ss : hs + ss + sl],
                                start=True,
                                stop=True,
                            )
                            ss += sl
                        lo = 0
                    else:
                        lo = rel
                        nc.tensor.matmul(
                            sp[:, rel : rel + 128],
                            lhsT=idn128_t,
                            rhs=maskt_t,
                            start=True,
                            stop=False,
                            skip_group_check=True,
                        )
                        nc.tensor.matmul(
                            sp[:, rel : rel + 128],
                            lhsT=xh[:, s0 : s0 + 128],
                            rhs=z[:, s0 : s0 + 128],
                            start=False,
                            stop=True,
                            skip_group_check=True,
                        )
                        ss = rel + 128
                        while ss < w:
                            sl = min(512 - (ss % 512), w - ss)
                            nc.tensor.matmul(
                                sp[:, ss : ss + sl],
                                lhsT=xh[:, s0 : s0 + 128],
                                rhs=z[:, hs + ss : hs + ss + sl],
                                start=True,
                                stop=True,
                            )
                            ss += sl
                    nc.scalar.activation(p[:, lo:w], sp[:, lo:w], AF.Exp)

                    # PV deferred by one j so PE computes S_{j+1} while the
                    # ACT engine exps j (avoids PE stalling on exp latency)
                    def _pv(j=j, p=p, rel=rel):
                        k0 = max(0, rel // 128)
                        for k in range(k0, nblk):
                            gi = t0 + k
                            # start zeroes the bank's whole 2KB zero region,
                            # so only the first block per bank carries it;
                            # the rest overwrite their pending-zero bytes
                            nc.tensor.matmul(
                                oa[k // 4][
                                    :, 128 * (k % 4) : 128 * (k % 4) + DK + 1
                                ],
                                lhsT=p[:, 128 * k : 128 * k + 128],
                                rhs=v3[:, j, :],
                                start=(j == 0 and k % 4 == 0),
                                stop=(j == gi),
                                skip_group_check=True,
                            )
                        kd = j - t0  # query block completing at this key block
                        if 0 <= kd < nblk and kd in drain_ends:
                            lo = max((e + 1 for e in drain_ends if e < kd), default=0)
                            _drain(lo, kd + 1)

                    # PV deferral is global: the previous key block's PV (and
                    # the previous CHUNK's tail PV+drains) are emitted after
                    # this block's scores+exp, so PE always has the next
                    # scores queued ahead of PV/drain work and ACT never
                    # starves at chunk boundaries
                    if pend_pv[0] is not None:
                        pend_pv[0]()
                    pend_pv[0] = _pv
                    if j == 0 and proj_rest is not None and hs == 0:
                        proj_rest()
                        pstate[pair][4] = None
                    if j == min(HOOKJ, njb - 1) and mid_hook is not None:
                        mid_hook()

            emit_proj(0)
            for pair in range(NP):
                for k, hs in enumerate(range(0, S_, 1024)):
                    # LN pops happen between chunks only, with enough drain
                    # age that the reduce DMA's data is certainly back -- a
                    # queue head must never sit waiting on a collective
                    _pop_ln(min_age=3)
                    hook = None
                    if k == 0 and pair + 1 < NP:
                        def hook(pr=pair):
                            emit_proj(pr + 1)
                            if pr + 2 < NP:
                                emit_loads((pr + 2) // HPC)
                    emit_chunk(pair, hs, mid_hook=hook)
            if pend_pv[0] is not None:
                pend_pv[0]()
                pend_pv[0] = None
            while pending_ln:
                _pop_ln(min_age=0)


    # Restrict Exp/Ln to the shared natural_log_exp_and_others table set so
    # the whole kernel uses one ACT table load (indices preserved).
    import concourse.bacc as _bacc_mod

    _orig_tables = _bacc_mod.get_activation_tables

    def _filtered_tables(arch):
        out = {}
        for name, fns in _orig_tables(arch).items():
            if name != "natural_log_exp_and_others":
                fns = set(fns) - {AF.Exp, AF.Ln}
            out[name] = fns
        return out

    _bacc_mod.get_activation_tables = _filtered_tables
    try:
        nc.compile()
    finally:
        _bacc_mod.get_activation_tables = _orig_tables
    return nc


_GRAPH_CACHE = {}


def _get_graph(apply_affine: bool) -> bass.Bass:
    if apply_affine not in _GRAPH_CACHE:
        _GRAPH_CACHE[apply_affine] = _build_graph(apply_affine)
    return _GRAPH_CACHE[apply_affine]


def _prep_in_maps(x, Wq, bq, Wk, bk, Wv, bv, gamma, beta, apply_affine):
    scale = 1.0 / np.sqrt(np.float32(DK))
    in_maps = []
    for i in range(NCORES):
        dsl = slice(DC * i, DC * (i + 1))
        hsl = slice(HPC * i, HPC * (i + 1))
        x_sl = x[:, :, dsl]
        xt_full = x_sl.transpose(0, 2, 1).reshape(x.shape[0], HPC, DK, x.shape[1])
        xt_aug = np.concatenate(
            [xt_full, np.ones((x.shape[0], HPC, 1, x.shape[1]), np.float32)], axis=2
        )
        Wq_s = (Wq[hsl] * scale).astype(np.float64)
        bq_s = (bq[hsl] * scale).astype(np.float64)
        Wk_h = Wk[hsl].astype(np.float64)
        bk_h = bk[hsl].astype(np.float64)
        M = np.einsum("hde,hfe->hdf", Wk_h, Wq_s)      # [h, dK, dQ]
        u = np.einsum("hde,he->hd", Wk_h, bq_s)        # alpha coeffs (per t)
        wvec = np.einsum("hde,he->hd", Wq_s, bk_h)     # beta coeffs (per s)
        cconst = np.einsum("he,he->h", bk_h, bq_s)
        # lhsT for z: [d', dK | wvec]; z rows 0..63 = M@xh^T + u, row 64 = xh.w + c
        zw_np = np.concatenate(
            [M.transpose(0, 2, 1), wvec[:, :, None]], axis=2
        )  # [h, d'(=dQ... contraction dim), dK+1]
        zb_np = np.concatenate([u, cconst[:, None]], axis=1)[:, :, None]
        m = {
            "xt": np.ascontiguousarray(xt_aug).astype(BF_NP),
            "xs": np.ascontiguousarray(x_sl),
            "wpack": np.ascontiguousarray(
                np.concatenate(
                    [zw_np[0], zw_np[1], Wv[hsl][0], Wv[hsl][1]], axis=1
                )
            ).astype(BF_NP),
            "zb": np.ascontiguousarray(zb_np[:, :, 0].T).astype(np.float32),
            "bv16": np.ascontiguousarray(
                np.tile(bv[hsl][:, None, :], (1, 128, 16))
            ).astype(np.float32),
        }
        if apply_affine:
            m["gam"] = np.ascontiguousarray(
                np.tile(gamma[dsl][None, :], (128, 1))
            ).astype(np.float32)
            m["bet"] = np.ascontiguousarray(
                np.tile(beta[dsl][None, :], (128, 1))
            ).astype(np.float32)
        in_maps.append(m)
    return in_maps


def kernel(x, Wq, bq, Wk, bk, Wv, bv, gamma, beta):
    global LAST_RESULTS
    x = np.asarray(x, np.float32)
    Wq = np.asarray(Wq, np.float32)
    bq = np.asarray(bq, np.float32)
    Wk = np.asarray(Wk, np.float32)
    bk = np.asarray(bk, np.float32)
    Wv = np.asarray(Wv, np.float32)
    bv = np.asarray(bv, np.float32)
    gamma = np.asarray(gamma, np.float32)
    beta = np.asarray(beta, np.float32)

    apply_affine = not (
        np.allclose(gamma, 1.0, atol=0.0, rtol=0.0)
        and np.allclose(beta, 0.0, atol=0.0, rtol=0.0)
    )
    fake_ar = bool(int(os.environ.get("KERNEL_FAKE_AR", "0")))
    nc = _get_graph(apply_affine) if not fake_ar else _build_graph(apply_affine, fake_ar=True)

    in_maps = _prep_in_maps(x, Wq, bq, Wk, bk, Wv, bv, gamma, beta, apply_affine)

    res = run_bass_kernel_spmd(
        nc,
        in_maps,
        core_ids=list(range(NCORES)),
        trace=bool(int(os.environ.get("KERNEL_TRACE", "0"))),
    )
    LAST_RESULTS = res
    outs = [np.asarray(r["out"], np.float32) for r in res.results]
    return np.concatenate(outs, axis=2)


if __name__ == "__main__":
    nc = _build_graph(False)
    print("graph built ok:", len(nc.inst_map), "instructions")

